# revision 4
# baseline (speedup 1.0000x reference)
"""Trainium2 Bass kernel for nn_AttnModel (BiAttn x3 + tiny FC + batch-softmax tile).

Contract: kernel(**inputs) takes the FULL inputs (a_emb/v_emb/l_emb [32,1024,32],
fc1_w [64,64], fc1_b [64], fc2_w [1,64]) and returns the FULL output [32,1024,64].

Sharding: data-parallel over batch across 8 cores (4 batches/core, 12
(batch,pair) "units"/core). Per unit only row 0 of each BiAttn output is
needed, which requires S = f@g^T [1024,1024], row/col sums of exp(S), and
row 0 / col 0 of exp(S):
  - S chunks [128i,1024j] via single-term fp16 matmuls (1 PE cycle/row;
    fp16 keeps ~2.4e-4 precision, same class as fp32r, half the DMA).
  - exp: ACT activation (bf16 out) for ~70% of chunks; DVE Schraudolph
    bit-trick exp (tensor_scalar S*A+B -> int16, bits == bf16(exp S),
    mean-calibrated B) for the rest. GPSIMD cannot touch PSUM, so Pool
    takes no exp work.
  - rowsums via DVE dummy tensor_scalar with accum_out (4x perf mode on
    bf16); colsum via per-chunk ones^T-bf16 matmuls accumulated in PSUM.
  - w1/w2 formed in j/i-partition layout via tiny PE transposes +
    reciprocal_approx_fast; o1/o2 via fp16 [128,1]x[128,32] matmuls.
  - tiny FC -> logits; ONE AllGather of exp(Ci) (15us cost-model constant)
    for the batch-dim softmax; Z-independent work precomputed before the
    collective; broadcast rows written as [1024,64] per batch.
Scheduling: emission is software-pipelined (colsum/rowsum of chunk c
deferred ~3 chunks; unit tails interleaved at chunk positions of the next
unit) to avoid head-of-line blocking in the in-order engine queues;
S triple-buffered in PSUM (3x2 banks + cs 1 + shared o/transpose bank 1).
"""
import numpy as np
import ml_dtypes

import concourse.bass as bass
import concourse.bacc as bacc
import concourse.tile as tile
import concourse.mybir as mybir
from concourse.bass_utils import run_bass_kernel_spmd
from concourse.tile_rust import add_dep_helper

F32 = mybir.dt.float32
F32R = mybir.dt.float32r
BF16 = mybir.dt.bfloat16
FP16 = mybir.dt.float16
I16 = mybir.dt.int16
AF = mybir.ActivationFunctionType
ALU = mybir.AluOpType

B, U, D = 32, 1024, 32
NCORES = 8
BPC = B // NCORES          # batches per core = 4
NU = 3 * BPC               # units per core = 12
NCH = U // 128             # i-chunks per unit = 8
PAIRS = [(0, 1), (0, 2), (1, 2)]

# Schraudolph exp->bf16 bit trick: bits = round(S * A + B); bf16(bits) ~ exp(S)
A_SCH = 128.0 / np.log(2.0)
B_SCH = 16256.0 - 7.365    # mean-centered on HW (round-to-nearest measured)

import os as _os
K_ACCUM = int(_os.environ.get("K_ACCUM", "0"))     # A-chunk rowsum: 1=ACT accum, 0=DVE dummy
K_NTREE = int(_os.environ.get("K_NTREE", "0"))     # units using add-tree colsum
K_NV = int(_os.environ.get("K_NV", "24"))          # DVE Schraudolph chunk count

# ---- engine assignment tables (tuning knobs) ----
# exp engine per (unit, chunk): 'A' = ACT activation (+accum rowsum),
# 'V' = DVE Schraudolph (+DVE dummy rowsum)
_VPOS = [(1, 4, 6), (2, 5), (3, 7), (1, 5, 7), (2, 6), (3, 6)]
_LASTV = int(_os.environ.get("K_LASTV", "0"))
def exp_eng(u, c):
    if _LASTV and u >= NU - _LASTV:
        return 'V' if c in (1, 3, 5, 7) else 'A'
    base3 = K_NV // 12
    extra = K_NV - 12 * base3
    nv = base3 + (1 if u % 12 < extra else 0)
    pos = _VPOS[u % len(_VPOS)]
    return 'V' if c in pos[:nv] else 'A'

# colsum route per unit: 'PE' = per-chunk ones-matmuls accumulated in PSUM,
# 'T' = bf16 add-tree (DVE/Pool) + one ones-matmul on the Esum
def cs_route(u):
    step = 12.0 / max(K_NTREE, 1e-9)
    marks = {int(i * step) for i in range(K_NTREE)}
    return 'T' if u in marks else 'PE'

# add-tree engines: level1 pairs, level2 pairs, level3
_POOL_ADDS = int(_os.environ.get("K_POOLADDS", "0"))
_EB = int(_os.environ.get("K_EB", "4"))
_DLAG = int(_os.environ.get("K_DLAG", "3"))
_TW = [int(x) for x in _os.environ.get("K_TW", "2,4,6,7").split(",")]
def add_eng(u, lvl):
    return 'P' if lvl < _POOL_ADDS else 'V'


def build_program(repeat=1):
    nc = bacc.Bacc("TRN2", target_bir_lowering=False, debug=False, num_devices=NCORES)

    emb16 = nc.dram_tensor("emb16", [3, 128, 2048], FP16, kind="ExternalInput")
    nb16 = nc.dram_tensor("nb16", [3, 128, 2048], FP16, kind="ExternalInput")
    f0g0 = nc.dram_tensor("f0g0", [3, 128, 64], F32, kind="ExternalInput")
    fc1T = nc.dram_tensor("fc1T", [64, 64], F32, kind="ExternalInput")
    fc1b = nc.dram_tensor("fc1b", [NU, 64], F32, kind="ExternalInput")
    fc2T = nc.dram_tensor("fc2T", [64, 1], F32, kind="ExternalInput")
    out = nc.dram_tensor("out", [BPC, U, 2 * D], F32, kind="ExternalOutput")

    ident_np = np.eye(128, dtype=np.float32)
    sel_np = np.zeros((NU, BPC), np.float32)
    for r in range(NU):
        sel_np[r, r // 3] = 1.0
    oh_np = np.zeros((BPC, BPC * 128), np.float16)
    for b in range(BPC):
        oh_np[b, 128 * b:128 * (b + 1)] = 1.0
    ksel_np = np.zeros((3, NU), np.float32)
    for r in range(NU):
        ksel_np[r % 3, r] = 1.0

    with tile.TileContext(nc) as tc:
        from contextlib import ExitStack
        ctx = ExitStack()
        consts = ctx.enter_context(tc.tile_pool(name="consts", bufs=1))
        bigp = ctx.enter_context(tc.tile_pool(name="big", bufs=1))
        epool = ctx.enter_context(tc.tile_pool(name="epool", bufs=1))
        upool = ctx.enter_context(tc.tile_pool(name="upool", bufs=2))
        tailp = ctx.enter_context(tc.tile_pool(name="tailp", bufs=1))
        dramp = ctx.enter_context(tc.tile_pool(name="dramp", bufs=1, space="DRAM"))

        sps = ctx.enter_context(tc.tile_pool(name="sps", bufs=1, space="PSUM"))
        csps = ctx.enter_context(tc.tile_pool(name="csps", bufs=1, space="PSUM"))
        otpp = ctx.enter_context(tc.tile_pool(name="otpp", bufs=1, space="PSUM"))

        # ---------------- input DMAs (first-needed first) ----------------
        emb, d_emb = [], []
        for p in range(3):
            t = bigp.tile([128, 2048], FP16, tag=f"emb{p}", name=f"emb_{p}")
            if p == 0:
                d0 = nc.sync.dma_start(t[0:32, :], emb16[p, 0:32, :])
                nc.sync.dma_start(t[32:128, :], emb16[p, 32:128, :])
                d_emb.append(d0)
            else:
                d_emb.append(nc.sync.dma_start(t[:], emb16[p, :, :]))
            emb.append(t)

        # ---------------- constants ----------------
        ident = consts.tile([128, 128], F32)
        d_ident = nc.sync.dma_start(ident[:], nc.inline_tensor(ident_np, name="c_ident")[:, :])
        ones_bf = consts.tile([128, 1], BF16)
        nc.vector.memset(ones_bf[:], 1.0)
        ones_f = consts.tile([128, 1], F32)
        nc.vector.memset(ones_f[:], 1.0)
        selT = consts.tile([NU, BPC], F32)
        d_sel = nc.sync.dma_start(selT[:], nc.inline_tensor(sel_np, name="c_sel")[:, :])
        oh = consts.tile([BPC, BPC * 128], FP16)
        d_oh = nc.sync.dma_start(oh[:], nc.inline_tensor(oh_np, name="c_oh")[:, :])
        fc1T_sb = consts.tile([64, 64], F32)
        d_fc1 = nc.sync.dma_start(fc1T_sb[:], fc1T[:, :])
        fc1b_sb = consts.tile([NU, 64], F32)
        d_fc1b = nc.sync.dma_start(fc1b_sb[:], fc1b[:, :])
        fc2T_sb = consts.tile([64, 1], F32)
        d_fc2 = nc.sync.dma_start(fc2T_sb[:], fc2T[:, :])
        ksel3 = consts.tile([3, NU], F32)
        d_ksel = nc.sync.dma_start(ksel3[:], nc.inline_tensor(ksel_np, name="c_ksel")[:, :])

        nb, d_nb, fg, d_fg = [], [], [], []
        for p in range(3):
            t = bigp.tile([128, 2048], FP16, tag=f"nb{p}", name=f"nb_{p}")
            d_nb.append(nc.sync.dma_start(t[:], nb16[p, :, :]))
            nb.append(t)
            t = bigp.tile([128, 64], F32, tag=f"fg{p}")
            d_fg.append(nc.sync.dma_start(t[:], f0g0[p, :, :]))
            fg.append(t)

        def guard(eng, deps):
            deps = [d for d in deps if d is not None]
            if not deps:
                return None
            n = eng.nop(nofuse=True)
            for d in deps:
                add_dep_helper(n.ins, d.ins, sync=True, reason="wait-carrier")
            return n

        def pin(inst, g):
            if g is not None:
                add_dep_helper(inst.ins, g.ins, sync=False, reason="order")

        biT_sb = tailp.tile([64, NU], F32)

        # one PSUM bank shared by o-accumulation (cols 0:64), e0 transposes
        # (64:80), cs transposes (80:96), bi transpose + FC tail (96:256)
        otp = otpp.tile([128, 256], F32, tag="otp", bufs=1, name="otp")

        # ---------------- per-unit state ----------------
        class Unit:
            pass

        units = []
        for u in range(NU):
            st = Unit()
            st.u = u
            st.p, st.t = u // 4, u % 4
            st.E = None
            st.rs = None
            units.append(st)

        from collections import deque
        defer2 = deque()

        def flush_defer(limit):
            while defer2 and defer2[0][0] <= limit:
                defer2.popleft()[1]()

        def emit_chunks(u):
            """S matmuls + exp + rowsum + tree adds for unit u."""
            st = units[u]
            p, t = st.p, st.t
            rb = 32 * t
            E = epool.tile([128, NCH * 1024], BF16, tag=f"E{u % _EB}", name=f"E_{u}")
            rs = upool.tile([128, NCH], F32, tag="rs", name=f"rs_{u}")
            st.E, st.rs = E, rs
            st.partials = {}
            eslc = emb[p]
            g0 = guard(nc.tensor, [d_emb[p]]) if t == 0 else None
            st.cs_ps = csps.tile([64, 512], F32, tag="cs", bufs=1, name=f"cs_{u}")
            for c in range(NCH):
                S_ps = sps.tile([128, 1024], F32, tag=f"S{(8 * u + c) % 3}", name=f"S_{u}_{c}")
                for h in range(2):
                    mm = nc.tensor.matmul(
                        S_ps[:, 512 * h:512 * (h + 1)],
                        eslc[rb:rb + 32, 128 * c:128 * (c + 1)],
                        eslc[rb:rb + 32, 1024 + 512 * h:1024 + 512 * (h + 1)],
                        start=True, stop=True, tile_position=(rb, 0))
                    if c == 0 and h == 0:
                        pin(mm, g0)
                # deferred work from 2 chunks ago keeps in-order queues unblocked
                while defer2 and defer2[0][0] <= 8 * u + c:
                    defer2.popleft()[1]()
                if u > 0:
                    if c == _TW[0]:
                        emit_wcols_a(u - 1)
                    elif c == _TW[1]:
                        emit_wcols_b(u - 1)
                    if c == _TW[2]:
                        emit_omm(u - 1)
                    if c == _TW[3] and (u - 1) % 4 == 3:
                        emit_pack_bi((u - 1) // 4)
                ec = E[:, 1024 * c:1024 * (c + 1)]
                if exp_eng(u, c) == 'A':
                    if K_ACCUM:
                        nc.scalar.activation(ec, S_ps[:], AF.Exp, accum_out=rs[:, c:c + 1])
                    else:
                        nc.scalar.activation(ec, S_ps[:], AF.Exp)
                else:
                    nc.vector.tensor_scalar(ec.bitcast(I16), S_ps[:], A_SCH, B_SCH,
                                            ALU.mult, ALU.add)

                def later(u=u, c=c, st=st, ec=ec, E=E, rs=rs):
                    if exp_eng(u, c) == 'V' or not K_ACCUM:
                        dummy = upool.tile([128, 1024], BF16, tag="dum", name=f"dum_{u}_{c}")
                        nc.vector.tensor_scalar(dummy[:], ec, 1.0, 0.0, ALU.mult, ALU.add,
                                                accum_out=rs[:, c:c + 1])
                    # colsum contribution of chunk c
                    if cs_route(u) == 'PE':
                        for h in range(2):
                            nc.tensor.matmul(st.cs_ps[32 * h:32 * h + 1, :], ones_bf[:, :],
                                             ec[:, 512 * h:512 * (h + 1)],
                                             start=(c == 0), stop=(c == NCH - 1),
                                             tile_position=(0, 32 * h))
                    else:
                        # bf16 add tree: P0..P3 = pairs, Q0,Q1, ES; adds on DVE/Pool
                        if c % 2 == 1:
                            l1 = c // 2
                            pl = upool.tile([128, 1024], BF16, tag=f"P{l1}", name=f"P{l1}_{u}")
                            eng = nc.gpsimd if add_eng(u, l1) == 'P' else nc.vector
                            eng.tensor_add(pl[:], E[:, 1024 * (c - 1):1024 * c], ec)
                            st.partials[f"P{l1}"] = pl
                            if l1 % 2 == 1:
                                l2 = l1 // 2
                                ql = upool.tile([128, 1024], BF16, tag=f"Q{l2}", name=f"Q{l2}_{u}")
                                eng = nc.gpsimd if add_eng(u, 4 + l2) == 'P' else nc.vector
                                eng.tensor_add(ql[:], st.partials[f"P{l1 - 1}"][:], pl[:])
                                st.partials[f"Q{l2}"] = ql
                        if c == NCH - 1:
                            es = upool.tile([128, 1024], BF16, tag="ES", name=f"ES_{u}")
                            eng = nc.gpsimd if add_eng(u, 6) == 'P' else nc.vector
                            eng.tensor_add(es[:], st.partials["Q0"][:], st.partials["Q1"][:])
                            for h in range(2):
                                nc.tensor.matmul(st.cs_ps[32 * h:32 * h + 1, :], ones_bf[:, :],
                                                 es[:, 512 * h:512 * (h + 1)],
                                                 start=True, stop=True,
                                                 tile_position=(0, 32 * h))
                    # e0 transposes: E row 0 (chunk 0 cols) -> j-partition columns
                    if c == 0:
                        e0v = otp[:, 64:80].bitcast(BF16)
                        for bblk in range(NCH):
                            nc.tensor.transpose(e0v[:, 2 * bblk:2 * bblk + 1],
                                                E[0:1, 128 * bblk:128 * (bblk + 1)],
                                                ones_bf[0:1, 0:1])
                        e0c = upool.tile([128, NCH], BF16, tag="e0c", name=f"e0c_{u}")
                        nc.vector.tensor_copy(e0c[:], e0v[:, 0:2 * NCH:2])
                        st.e0c = e0c
                defer2.append((8 * u + c + _DLAG, later))

        def emit_wcols_a(u):
            st = units[u]
            cs_sb = upool.tile([64, 512], F32, tag="cssb", name=f"cssb_{u}")
            nc.vector.tensor_copy(cs_sb[0:1, :], st.cs_ps[0:1, :])
            nc.scalar.copy(cs_sb[32:33, :], st.cs_ps[32:33, :])
            st.cs_sb = cs_sb
            # rowsums -> w2
            rsr = upool.tile([128, NCH], F32, tag="rsr", name=f"rsr_{u}")
            nc.vector.reciprocal_approx_fast(rsr[:], st.rs[:])
            w2 = upool.tile([128, NCH], FP16, tag="w2", name=f"w2_{u}")
            nc.vector.tensor_mul(w2[:], st.E[:, 0:NCH * 1024:1024], rsr[:])
            st.w2 = w2

        def emit_wcols_b(u):
            st = units[u]
            cs_sb = st.cs_sb
            cstp = otp[:, 80:96]
            for h in range(2):
                for bblk in range(4):
                    nc.tensor.transpose(
                        cstp[:, 4 * h + bblk:4 * h + bblk + 1],
                        cs_sb[32 * h:32 * h + 1, 128 * bblk:128 * (bblk + 1)],
                        ones_f[32 * h:32 * h + 1, 0:1])
            # cstp col order: j = 512h + 128b + part = chunk-major col (c = 4h+b)
            crec = upool.tile([128, NCH], F32, tag="crec", name=f"crec_{u}")
            nc.vector.reciprocal_approx_fast(crec[:], cstp[:, 0:NCH])
            w1 = upool.tile([128, NCH], FP16, tag="w1", name=f"w1_{u}")
            nc.vector.tensor_mul(w1[:], st.e0c[:], crec[:])
            st.w1 = w1

        def emit_omm(u):
            st = units[u]
            p, t = st.p, st.t
            o_ps = otp
            for c in range(NCH):
                # o1 = sum_j w1_j g_j   (gN at cols 512t+256..512)
                nc.tensor.matmul(
                    o_ps[32 * t:32 * t + 1, 0:32],
                    st.w1[:, c:c + 1],
                    nb[p][:, 512 * t + 256 + 32 * c:512 * t + 256 + 32 * (c + 1)],
                    start=(c == 0), stop=(c == NCH - 1), tile_position=(0, 32 * t))
            for c in range(NCH):
                # o2 = sum_i w2_i f_i   (fN at cols 512t..256)
                nc.tensor.matmul(
                    o_ps[32 * t:32 * t + 1, 32:64],
                    st.w2[:, c:c + 1],
                    nb[p][:, 512 * t + 32 * c:512 * t + 32 * (c + 1)],
                    start=(c == 0), stop=(c == NCH - 1), tile_position=(0, 32 * t))

        def emit_pack_bi(p):
            bi_rows = upool.tile([128, 64], F32, tag="bi", name=f"bi_{p}")
            nc.vector.tensor_mul(bi_rows[:], otp[:, 0:64], fg[p][:])
            tpb_ps = otp[0:64, 96:224]
            nc.tensor.transpose(tpb_ps, bi_rows[:, 0:64], ident[:])
            nc.vector.tensor_copy(biT_sb[:, 4 * p:4 * (p + 1)], tpb_ps[:, 0:97:32])

        # ---------------- main pipeline ----------------
        # tail pieces of unit u-1 are emitted at chunk positions of unit u
        for u in range(NU):
            emit_chunks(u)
        flush_defer(10 ** 9)
        emit_wcols_a(NU - 1)
        emit_wcols_b(NU - 1)
        emit_omm(NU - 1)
        emit_pack_bi(2)

        # ---------------- tail: FC + single AllGather + batch softmax ----------------
        gt1 = guard(nc.tensor, [d_fc1, d_fc1b, d_fc2, d_sel, d_oh, d_ksel, d_ident])
        h_ps = otp[0:NU, 96:160]
        mmh = nc.tensor.matmul(h_ps, biT_sb[:], fc1T_sb[:], start=True, stop=True)
        pin(mmh, gt1)
        hb = tailp.tile([NU, 64], F32)
        nc.vector.tensor_add(hb[:], h_ps, fc1b_sb[:])
        hth = tailp.tile([NU, 64], F32)
        nc.scalar.activation(hth[:], hb[:], AF.Tanh)
        hT_ps = otp[0:64, 160:172]
        nc.tensor.transpose(hT_ps, hth[:], ident[0:12, 0:12])
        hT = tailp.tile([64, NU], F32)
        nc.vector.tensor_copy(hT[:], hT_ps)
        ci_ps = otp[0:NU, 176:177]
        nc.tensor.matmul(ci_ps, hT[:], fc2T_sb[:], start=True, stop=True)
        eci = tailp.tile([NU, 1], F32)
        nc.scalar.activation(eci[:], ci_ps, AF.Exp)

        # Z-independent pre-collective work: bi12 rows and M = eci * Bi
        bi12_ps = otp[0:NU, 180:244]
        nc.tensor.transpose(bi12_ps, biT_sb[:], ident[0:64, 0:64])
        bi12 = tailp.tile([NU, 64], F32)
        nc.vector.tensor_copy(bi12[:], bi12_ps)
        M = tailp.tile([NU, 64], F32)
        nc.vector.tensor_scalar_mul(M[:], bi12[:], eci[:])

        cc_in = dramp.tile([NU, 1], F32, name="cc_in")
        cc_out = dramp.tile([NCORES * NU, 1], F32, name="cc_out")
        nc.sync.dma_start(cc_in[:], eci[:])
        # keep the PE p-state hot through the collective so the final
        # broadcast matmuls run at full clock (filler writes to a dead tile)
        NJUNK = int(_os.environ.get("K_NJUNK", "70"))
        if NJUNK:
            junk_ps = csps.tile([64, 512], F32, tag="cs", bufs=1, name="junk_ps")
            for jj in range(NJUNK):
                nc.tensor.matmul(junk_ps[0:1, :], ones_bf[:, :],
                                 emb[0][:, 0:512].bitcast(BF16),
                                 start=True, stop=True)
        nc.gpsimd.collective_compute(
            "AllGather", ALU.bypass,
            replica_groups=[list(range(NCORES))],
            ins=[cc_in.opt()], outs=[cc_out.opt()],
        )
        zl = tailp.tile([3, NCORES * BPC], F32, name="zl")
        nc.sync.dma_start(zl[:], bass.AP(cc_out[:].tensor, 0, [[1, 3], [3, NCORES * BPC]]))
        zk = tailp.tile([3, 1], F32, name="zk")
        nc.vector.reduce_sum(zk[:], zl[:], axis=mybir.AxisListType.X)
        zcol_ps = otp[0:NU, 248:249]
        mmz = nc.tensor.matmul(zcol_ps, ksel3[:], zk[:], start=True, stop=True)
        zr = tailp.tile([NU, 1], F32)
        nc.vector.reciprocal(zr[:], zcol_ps)
        M2 = tailp.tile([NU, 64], F32)
        nc.vector.tensor_scalar_mul(M2[:], M[:], zr[:])
        rows_ps = otp[0:BPC, 96:160]
        nc.tensor.matmul(rows_ps, selT[:], M2[:], start=True, stop=True)
        rows_sb = tailp.tile([BPC, 64], FP16)
        nc.vector.tensor_copy(rows_sb[:], rows_ps)
        rep = tailp.tile([BPC, 512], FP16)
        nc.vector.tensor_copy(
            rep[:].rearrange("p (r d) -> p r d", r=8),
            rows_sb[:, None, :].broadcast_to([BPC, 8, 64]),
        )
        for b in range(BPC):
            bc_ps = csps.tile([128, 512], F32, tag="cs", bufs=1, name=f"bc_ps{b}")
            nc.tensor.matmul(bc_ps[:], oh[:, 128 * b:128 * (b + 1)], rep[:],
                             start=True, stop=True)
            bc_sb = tailp.tile([128, 512], F32, tag=f"bc{b}")
            if b % 2 == 0:
                nc.vector.tensor_copy(bc_sb[:], bc_ps[:])
            else:
                nc.scalar.copy(bc_sb[:], bc_ps[:])
            nc.sync.dma_start(
                out[b].rearrange("(p r) d -> p (r d)", p=128),
                bc_sb[:],
            )
        ctx.close()
    nc.finalize()
    return nc


def make_in_maps(a_emb, v_emb, l_emb, fc1_w, fc1_b, fc2_w):
    embs = [a_emb, v_emb, l_emb]
    fc1T = np.ascontiguousarray(fc1_w.T, np.float32)
    fc1b = np.ascontiguousarray(np.tile(fc1_b[None, :], (NU, 1)), np.float32)
    fc2T = np.ascontiguousarray(fc2_w.T, np.float32)
    in_maps = []
    for core in range(NCORES):
        emb16 = np.zeros((3, 128, 2048), np.float16)
        nb16 = np.zeros((3, 128, 2048), np.float16)
        f0g0 = np.zeros((3, 128, 64), np.float32)
        for u in range(NU):
            p, t = u // 4, u % 4
            b = BPC * core + u // 3
            fi, gi = PAIRS[u % 3]
            f = np.asarray(embs[fi][b], np.float32)   # [1024, 32]
            g = np.asarray(embs[gi][b], np.float32)
            rb = 32 * t
            emb16[p, rb:rb + 32, 0:1024] = f.T.astype(np.float16)
            emb16[p, rb:rb + 32, 1024:2048] = g.T.astype(np.float16)
            fN = f.reshape(NCH, 128, D).transpose(1, 0, 2).reshape(128, NCH * D)
            gN = g.reshape(NCH, 128, D).transpose(1, 0, 2).reshape(128, NCH * D)
            nb16[p, :, 512 * t:512 * t + 256] = fN.astype(np.float16)
            nb16[p, :, 512 * t + 256:512 * t + 512] = gN.astype(np.float16)
            f0g0[p, rb, 0:32] = f[0]
            f0g0[p, rb, 32:64] = g[0]
        in_maps.append({
            "emb16": emb16, "nb16": nb16, "f0g0": f0g0,
            "fc1T": fc1T, "fc1b": fc1b, "fc2T": fc2T,
        })
    return in_maps


_PROGRAM_CACHE = {}


def _get_program(repeat=1):
    key = ("nc", repeat)
    if key not in _PROGRAM_CACHE:
        _PROGRAM_CACHE[key] = build_program(repeat)
    return _PROGRAM_CACHE[key]


def kernel(a_emb, v_emb, l_emb, fc1_w, fc1_b, fc2_w, _want_results=False):
    a_emb = np.asarray(a_emb, np.float32)
    v_emb = np.asarray(v_emb, np.float32)
    l_emb = np.asarray(l_emb, np.float32)
    fc1_w = np.asarray(fc1_w, np.float32)
    fc1_b = np.asarray(fc1_b, np.float32)
    fc2_w = np.asarray(fc2_w, np.float32)
    nc = _get_program()
    in_maps = make_in_maps(a_emb, v_emb, l_emb, fc1_w, fc1_b, fc2_w)
    res = None
    for attempt in range(3):
        try:
            res = run_bass_kernel_spmd(nc, in_maps, core_ids=list(range(NCORES)))
            break
        except Exception:
            if attempt == 2:
                raise
    assert res is not None
    outp = np.concatenate([res.results[c]["out"] for c in range(NCORES)], axis=0)
    if _want_results:
        return outp, res
    return outp


# revision 7
# speedup vs baseline: 1.0425x; 1.0425x over previous
"""Trainium2 Bass kernel for nn_AttnModel (BiAttn x3 + tiny FC + batch-softmax tile).

Contract: kernel(**inputs) takes the FULL inputs (a_emb/v_emb/l_emb [32,1024,32],
fc1_w [64,64], fc1_b [64], fc2_w [1,64]) and returns the FULL output [32,1024,64].

Sharding: data-parallel over batch across 8 cores (4 batches/core, 12
(batch,pair) "units"/core). Per unit only row 0 of each BiAttn output is
needed, which requires S = f@g^T [1024,1024], row/col sums of exp(S), and
row 0 / col 0 of exp(S):
  - S chunks [128i,1024j] via single-term fp16 matmuls (1 PE cycle/row;
    fp16 keeps ~2.4e-4 precision, same class as fp32r, half the DMA).
  - exp: ACT activation (bf16 out) for ~70% of chunks; DVE Schraudolph
    bit-trick exp (tensor_scalar S*A+B -> int16, bits == bf16(exp S),
    mean-calibrated B) for the rest. GPSIMD cannot touch PSUM, so Pool
    takes no exp work.
  - rowsums via DVE dummy tensor_scalar with accum_out (4x perf mode on
    bf16); colsum via per-chunk ones^T-bf16 matmuls accumulated in PSUM.
  - w1/w2 formed in j/i-partition layout via tiny PE transposes +
    reciprocal_approx_fast; o1/o2 via fp16 [128,1]x[128,32] matmuls.
  - tiny FC -> logits; ONE AllGather of exp(Ci) (15us cost-model constant)
    for the batch-dim softmax; Z-independent work precomputed before the
    collective; broadcast rows written as [1024,64] per batch.
Scheduling: emission is software-pipelined (colsum/rowsum of chunk c
deferred ~3 chunks; unit tails interleaved at chunk positions of the next
unit) to avoid head-of-line blocking in the in-order engine queues;
S triple-buffered in PSUM (3x2 banks + cs 1 + shared o/transpose bank 1).
"""
import numpy as np
import ml_dtypes

import concourse.bass as bass
import concourse.bacc as bacc
import concourse.tile as tile
import concourse.mybir as mybir
from concourse.bass_utils import run_bass_kernel_spmd
from concourse.tile_rust import add_dep_helper

F32 = mybir.dt.float32
F32R = mybir.dt.float32r
BF16 = mybir.dt.bfloat16
FP16 = mybir.dt.float16
I16 = mybir.dt.int16
AF = mybir.ActivationFunctionType
ALU = mybir.AluOpType

B, U, D = 32, 1024, 32
NCORES = 8
BPC = B // NCORES          # batches per core = 4
NU = 3 * BPC               # units per core = 12
NCH = U // 128             # i-chunks per unit = 8
PAIRS = [(0, 1), (0, 2), (1, 2)]

# Schraudolph exp->bf16 bit trick: bits = round(S * A + B); bf16(bits) ~ exp(S)
A_SCH = 128.0 / np.log(2.0)
B_SCH = 16256.0 - 7.365    # mean-centered on HW (round-to-nearest measured)

import os as _os
K_ACCUM = int(_os.environ.get("K_ACCUM", "0"))     # A-chunk rowsum: 1=ACT accum, 0=DVE dummy
K_NTREE = int(_os.environ.get("K_NTREE", "0"))     # units using add-tree colsum
K_NV = int(_os.environ.get("K_NV", "24"))          # DVE Schraudolph chunk count

# ---- engine assignment tables (tuning knobs) ----
# exp engine per (unit, chunk): 'A' = ACT activation (+accum rowsum),
# 'V' = DVE Schraudolph (+DVE dummy rowsum)
_VPOS_TABLES = {
    0: [(1, 4, 6), (2, 5), (3, 7), (1, 5, 7), (2, 6), (3, 6)],
    1: [(2, 5)] * 6,
    2: [(2, 6), (3, 5), (2, 6), (3, 5), (2, 6), (3, 5)],
    3: [(1, 5), (2, 6), (3, 7), (1, 5), (2, 6), (3, 7)],
}
_VPOS = _VPOS_TABLES[int(_os.environ.get("K_VT", "0"))]
_LASTV = int(_os.environ.get("K_LASTV", "0"))
def exp_eng(u, c):
    if _LASTV and u >= NU - _LASTV:
        return 'V' if c in (1, 3, 5, 7) else 'A'
    base3 = K_NV // 12
    extra = K_NV - 12 * base3
    nv = base3 + (1 if u % 12 < extra else 0)
    pos = _VPOS[u % len(_VPOS)]
    return 'V' if c in pos[:nv] else 'A'

# colsum route per unit: 'PE' = per-chunk ones-matmuls accumulated in PSUM,
# 'T' = bf16 add-tree (DVE/Pool) + one ones-matmul on the Esum
def cs_route(u):
    step = 12.0 / max(K_NTREE, 1e-9)
    marks = {int(i * step) for i in range(K_NTREE)}
    return 'T' if u in marks else 'PE'

# add-tree engines: level1 pairs, level2 pairs, level3
_POOL_ADDS = int(_os.environ.get("K_POOLADDS", "0"))
_EB = int(_os.environ.get("K_EB", "4"))
_DLAG = int(_os.environ.get("K_DLAG", "3"))
_TW = [int(x) for x in _os.environ.get("K_TW", "2,4,6,7").split(",")]
def add_eng(u, lvl):
    return 'P' if lvl < _POOL_ADDS else 'V'


def build_program(repeat=1):
    nc = bacc.Bacc("TRN2", target_bir_lowering=False, debug=False, num_devices=NCORES)

    emb16 = nc.dram_tensor("emb16", [3, 128, 2048], FP16, kind="ExternalInput")
    nb16 = nc.dram_tensor("nb16", [3, 128, 2048], FP16, kind="ExternalInput")
    f0g0 = nc.dram_tensor("f0g0", [3, 128, 64], F32, kind="ExternalInput")
    fc1T = nc.dram_tensor("fc1T", [64, 64], F32, kind="ExternalInput")
    fc1b = nc.dram_tensor("fc1b", [64, 1], F32, kind="ExternalInput")
    fc2T = nc.dram_tensor("fc2T", [64, 1], F32, kind="ExternalInput")
    out = nc.dram_tensor("out", [BPC, U, 2 * D], F32, kind="ExternalOutput")

    ident_np = np.eye(128, dtype=np.float32)
    sel_np = np.zeros((NU, BPC), np.float32)
    for r in range(NU):
        sel_np[r, r // 3] = 1.0
    oh_np = np.zeros((BPC, BPC * 128), np.float16)
    for b in range(BPC):
        oh_np[b, 128 * b:128 * (b + 1)] = 1.0
    ksel_np = np.zeros((3, NU), np.float32)
    for r in range(NU):
        ksel_np[r % 3, r] = 1.0

    with tile.TileContext(nc) as tc:
        from contextlib import ExitStack
        ctx = ExitStack()
        consts = ctx.enter_context(tc.tile_pool(name="consts", bufs=1))
        bigp = ctx.enter_context(tc.tile_pool(name="big", bufs=1))
        epool = ctx.enter_context(tc.tile_pool(name="epool", bufs=1))
        upool = ctx.enter_context(tc.tile_pool(name="upool", bufs=2))
        tailp = ctx.enter_context(tc.tile_pool(name="tailp", bufs=1))
        dramp = ctx.enter_context(tc.tile_pool(name="dramp", bufs=1, space="DRAM"))

        sps = ctx.enter_context(tc.tile_pool(name="sps", bufs=1, space="PSUM"))
        csps = ctx.enter_context(tc.tile_pool(name="csps", bufs=1, space="PSUM"))
        otpp = ctx.enter_context(tc.tile_pool(name="otpp", bufs=1, space="PSUM"))

        # ---------------- input DMAs (first-needed first) ----------------
        emb, d_emb = [], []
        for p in range(3):
            t = bigp.tile([128, 2048], FP16, tag=f"emb{p}", name=f"emb_{p}")
            if p == 0:
                d0 = nc.sync.dma_start(t[0:32, :], emb16[p, 0:32, :])
                nc.sync.dma_start(t[32:64, :], emb16[p, 32:64, :])
                nc.sync.dma_start(t[64:128, :], emb16[p, 64:128, :])
                d_emb.append(d0)
            else:
                d_emb.append(nc.sync.dma_start(t[:], emb16[p, :, :]))
            emb.append(t)

        # ---------------- constants (memsets) + data DMAs in first-use order ----
        ones_bf = consts.tile([128, 1], BF16)
        nc.vector.memset(ones_bf[:], 1.0)
        ones_f = consts.tile([128, 1], F32)
        nc.vector.memset(ones_f[:], 1.0)

        nb, d_nb, fg, d_fg = [], [], [], []
        for p in range(3):
            t = bigp.tile([128, 2048], FP16, tag=f"nb{p}", name=f"nb_{p}")
            d_nb.append(nc.sync.dma_start(t[:], nb16[p, :, :]))
            nb.append(t)
            t = bigp.tile([128, 64], F32, tag=f"fg{p}")
            d_fg.append(nc.sync.dma_start(t[:], f0g0[p, :, :]))
            fg.append(t)

        ident = consts.tile([128, 128], F32)
        d_ident = nc.sync.dma_start(ident[:], nc.inline_tensor(ident_np, name="c_ident")[:, :])
        selT = consts.tile([NU, BPC], F32)
        d_sel = nc.sync.dma_start(selT[:], nc.inline_tensor(sel_np, name="c_sel")[:, :])
        oh = consts.tile([BPC, BPC * 128], FP16)
        d_oh = nc.sync.dma_start(oh[:], nc.inline_tensor(oh_np, name="c_oh")[:, :])
        fc1T_sb = consts.tile([64, 64], F32)
        d_fc1 = nc.sync.dma_start(fc1T_sb[:], fc1T[:, :])
        fc1b_sb = consts.tile([64, 1], F32)
        d_fc1b = nc.sync.dma_start(fc1b_sb[:], fc1b[:, :])
        fc2T_sb = consts.tile([64, 1], F32)
        d_fc2 = nc.sync.dma_start(fc2T_sb[:], fc2T[:, :])
        ksel3 = consts.tile([3, NU], F32)
        d_ksel = nc.sync.dma_start(ksel3[:], nc.inline_tensor(ksel_np, name="c_ksel")[:, :])

        def guard(eng, deps):
            deps = [d for d in deps if d is not None]
            if not deps:
                return None
            n = eng.nop(nofuse=True)
            for d in deps:
                add_dep_helper(n.ins, d.ins, sync=True, reason="wait-carrier")
            return n

        def pin(inst, g):
            if g is not None:
                add_dep_helper(inst.ins, g.ins, sync=False, reason="order")

        biT_sb = tailp.tile([64, NU], F32)

        # one PSUM bank shared by o-accumulation (cols 0:64), e0 transposes
        # (64:80), cs transposes (80:96), bi transpose + FC tail (96:256)
        otp = otpp.tile([128, 256], F32, tag="otp", bufs=1, name="otp")

        # ---------------- per-unit state ----------------
        class Unit:
            pass

        units = []
        for u in range(NU):
            st = Unit()
            st.u = u
            st.p, st.t = u // 4, u % 4
            st.E = None
            st.rs = None
            units.append(st)

        from collections import deque
        defer2 = deque()

        def flush_defer(limit):
            while defer2 and defer2[0][0] <= limit:
                defer2.popleft()[1]()

        def emit_chunks(u):
            """S matmuls + exp + rowsum + tree adds for unit u."""
            st = units[u]
            p, t = st.p, st.t
            rb = 32 * t
            E = epool.tile([128, NCH * 1024], BF16, tag=f"E{u % _EB}", name=f"E_{u}")
            rs = upool.tile([128, NCH], F32, tag="rs", name=f"rs_{u}")
            st.E, st.rs = E, rs
            st.partials = {}
            eslc = emb[p]
            g0 = guard(nc.tensor, [d_emb[p]]) if t == 0 else None
            st.cs_ps = csps.tile([64, 512], F32, tag="cs", bufs=1, name=f"cs_{u}")
            for c in range(NCH):
                S_ps = sps.tile([128, 1024], F32, tag=f"S{(8 * u + c) % 3}", name=f"S_{u}_{c}")
                for h in range(2):
                    mm = nc.tensor.matmul(
                        S_ps[:, 512 * h:512 * (h + 1)],
                        eslc[rb:rb + 32, 128 * c:128 * (c + 1)],
                        eslc[rb:rb + 32, 1024 + 512 * h:1024 + 512 * (h + 1)],
                        start=True, stop=True, tile_position=(rb, 0))
                    if c == 0 and h == 0:
                        pin(mm, g0)
                # deferred work from 2 chunks ago keeps in-order queues unblocked
                while defer2 and defer2[0][0] <= 8 * u + c:
                    defer2.popleft()[1]()
                if u > 0:
                    if c == _TW[0]:
                        emit_wcols_a(u - 1)
                    elif c == _TW[1]:
                        emit_wcols_b(u - 1)
                    if c == _TW[2]:
                        emit_omm(u - 1)
                    if c == _TW[3] and (u - 1) % 4 == 3:
                        emit_pack_bi((u - 1) // 4)
                ec = E[:, 1024 * c:1024 * (c + 1)]
                if exp_eng(u, c) == 'A':
                    if K_ACCUM:
                        nc.scalar.activation(ec, S_ps[:], AF.Exp, accum_out=rs[:, c:c + 1])
                    else:
                        nc.scalar.activation(ec, S_ps[:], AF.Exp)
                else:
                    nc.vector.tensor_scalar(ec.bitcast(I16), S_ps[:], A_SCH, B_SCH,
                                            ALU.mult, ALU.add)

                def later(u=u, c=c, st=st, ec=ec, E=E, rs=rs):
                    if exp_eng(u, c) == 'V' or not K_ACCUM:
                        dummy = upool.tile([128, 1024], BF16, tag="dum", name=f"dum_{u}_{c}")
                        nc.vector.tensor_scalar(dummy[:], ec, 1.0, 0.0, ALU.mult, ALU.add,
                                                accum_out=rs[:, c:c + 1])
                    # colsum contribution of chunk c
                    if cs_route(u) == 'PE':
                        for h in range(2):
                            nc.tensor.matmul(st.cs_ps[32 * h:32 * h + 1, :], ones_bf[:, :],
                                             ec[:, 512 * h:512 * (h + 1)],
                                             start=(c == 0), stop=(c == NCH - 1),
                                             tile_position=(0, 32 * h))
                    else:
                        # bf16 add tree: P0..P3 = pairs, Q0,Q1, ES; adds on DVE/Pool
                        if c % 2 == 1:
                            l1 = c // 2
                            pl = upool.tile([128, 1024], BF16, tag=f"P{l1}", name=f"P{l1}_{u}")
                            eng = nc.gpsimd if add_eng(u, l1) == 'P' else nc.vector
                            eng.tensor_add(pl[:], E[:, 1024 * (c - 1):1024 * c], ec)
                            st.partials[f"P{l1}"] = pl
                            if l1 % 2 == 1:
                                l2 = l1 // 2
                                ql = upool.tile([128, 1024], BF16, tag=f"Q{l2}", name=f"Q{l2}_{u}")
                                eng = nc.gpsimd if add_eng(u, 4 + l2) == 'P' else nc.vector
                                eng.tensor_add(ql[:], st.partials[f"P{l1 - 1}"][:], pl[:])
                                st.partials[f"Q{l2}"] = ql
                        if c == NCH - 1:
                            es = upool.tile([128, 1024], BF16, tag="ES", name=f"ES_{u}")
                            eng = nc.gpsimd if add_eng(u, 6) == 'P' else nc.vector
                            eng.tensor_add(es[:], st.partials["Q0"][:], st.partials["Q1"][:])
                            for h in range(2):
                                nc.tensor.matmul(st.cs_ps[32 * h:32 * h + 1, :], ones_bf[:, :],
                                                 es[:, 512 * h:512 * (h + 1)],
                                                 start=True, stop=True,
                                                 tile_position=(0, 32 * h))
                    # e0 transposes: E row 0 (chunk 0 cols) -> j-partition columns
                    if c == 0:
                        e0v = otp[:, 64:80].bitcast(BF16)
                        for bblk in range(NCH):
                            nc.tensor.transpose(e0v[:, 2 * bblk:2 * bblk + 1],
                                                E[0:1, 128 * bblk:128 * (bblk + 1)],
                                                ones_bf[0:1, 0:1])
                        e0c = upool.tile([128, NCH], BF16, tag="e0c", name=f"e0c_{u}")
                        nc.vector.tensor_copy(e0c[:], e0v[:, 0:2 * NCH:2])
                        st.e0c = e0c
                defer2.append((8 * u + c + _DLAG, later))

        def emit_wcols_a(u):
            st = units[u]
            cs_sb = upool.tile([64, 512], F32, tag="cssb", name=f"cssb_{u}")
            nc.vector.tensor_copy(cs_sb[0:1, :], st.cs_ps[0:1, :])
            nc.scalar.copy(cs_sb[32:33, :], st.cs_ps[32:33, :])
            st.cs_sb = cs_sb
            # rowsums -> w2
            rsr = upool.tile([128, NCH], F32, tag="rsr", name=f"rsr_{u}")
            nc.vector.reciprocal_approx_fast(rsr[:], st.rs[:])
            w2 = upool.tile([128, NCH], FP16, tag="w2", name=f"w2_{u}")
            nc.vector.tensor_mul(w2[:], st.E[:, 0:NCH * 1024:1024], rsr[:])
            st.w2 = w2

        def emit_wcols_b(u):
            st = units[u]
            cs_sb = st.cs_sb
            cstp = otp[:, 80:96]
            for h in range(2):
                for bblk in range(4):
                    nc.tensor.transpose(
                        cstp[:, 4 * h + bblk:4 * h + bblk + 1],
                        cs_sb[32 * h:32 * h + 1, 128 * bblk:128 * (bblk + 1)],
                        ones_f[32 * h:32 * h + 1, 0:1])
            # cstp col order: j = 512h + 128b + part = chunk-major col (c = 4h+b)
            crec = upool.tile([128, NCH], F32, tag="crec", name=f"crec_{u}")
            nc.vector.reciprocal_approx_fast(crec[:], cstp[:, 0:NCH])
            w1 = upool.tile([128, NCH], FP16, tag="w1", name=f"w1_{u}")
            nc.vector.tensor_mul(w1[:], st.e0c[:], crec[:])
            st.w1 = w1

        def emit_omm(u):
            st = units[u]
            p, t = st.p, st.t
            o_ps = otp
            for c in range(NCH):
                # o1 = sum_j w1_j g_j   (gN at cols 512t+256..512)
                nc.tensor.matmul(
                    o_ps[32 * t:32 * t + 1, 0:32],
                    st.w1[:, c:c + 1],
                    nb[p][:, 512 * t + 256 + 32 * c:512 * t + 256 + 32 * (c + 1)],
                    start=(c == 0), stop=(c == NCH - 1), tile_position=(0, 32 * t))
            for c in range(NCH):
                # o2 = sum_i w2_i f_i   (fN at cols 512t..256)
                nc.tensor.matmul(
                    o_ps[32 * t:32 * t + 1, 32:64],
                    st.w2[:, c:c + 1],
                    nb[p][:, 512 * t + 32 * c:512 * t + 32 * (c + 1)],
                    start=(c == 0), stop=(c == NCH - 1), tile_position=(0, 32 * t))

        def emit_pack_bi(p):
            bi_rows = upool.tile([128, 64], F32, tag="bi", name=f"bi_{p}")
            nc.vector.tensor_mul(bi_rows[:], otp[:, 0:64], fg[p][:])
            tpb_ps = otp[0:64, 96:224]
            nc.tensor.transpose(tpb_ps, bi_rows[:, 0:64], ident[:])
            nc.vector.tensor_copy(biT_sb[:, 4 * p:4 * (p + 1)], tpb_ps[:, 0:97:32])

        # ---------------- main pipeline ----------------
        # tail pieces of unit u-1 are emitted at chunk positions of unit u
        for u in range(NU):
            emit_chunks(u)
        flush_defer(10 ** 9)
        emit_wcols_a(NU - 1)
        emit_wcols_b(NU - 1)
        emit_omm(NU - 1)
        emit_pack_bi(2)

        # ---------------- tail: FC + single AllGather + batch softmax ----------------
        gt1 = guard(nc.tensor, [d_fc1, d_fc1b, d_fc2, d_sel, d_oh, d_ksel, d_ident])
        hT_ps = otp[0:64, 160:172]
        mmh = nc.tensor.matmul(hT_ps, fc1T_sb[:], biT_sb[:], start=True, stop=True)
        pin(mmh, gt1)
        htT = tailp.tile([64, NU], F32)
        nc.scalar.activation(htT[:], hT_ps, AF.Tanh, bias=fc1b_sb[:, 0:1])
        ci_ps = otp[0:1, 176:188]
        nc.tensor.matmul(ci_ps, fc2T_sb[:], htT[:], start=True, stop=True)
        eci_r = tailp.tile([1, NU], F32)
        nc.scalar.activation(eci_r[:], ci_ps, AF.Exp)
        # eci as a [12,1] column for the M product (off the collective path)
        eciT_ps = otp[0:NU, 190:191]
        nc.tensor.transpose(eciT_ps, eci_r[:], ones_f[0:1, 0:1])
        eci = tailp.tile([NU, 1], F32)
        nc.vector.tensor_copy(eci[:], eciT_ps)

        # Z-independent pre-collective work: bi12 rows and M = eci * Bi
        bi12_ps = otp[0:NU, 180:244]
        nc.tensor.transpose(bi12_ps, biT_sb[:], ident[0:64, 0:64])
        bi12 = tailp.tile([NU, 64], F32)
        nc.vector.tensor_copy(bi12[:], bi12_ps)
        M = tailp.tile([NU, 64], F32)
        nc.vector.tensor_scalar_mul(M[:], bi12[:], eci[:])

        cc_in = dramp.tile([1, NU], F32, name="cc_in")
        cc_out = dramp.tile([1, NCORES * NU], F32, name="cc_out")
        nc.sync.dma_start(cc_in[:], eci_r[:])
        # keep the PE p-state hot through the collective so the final
        # broadcast matmuls run at full clock (filler writes to a dead tile)
        NJUNK = int(_os.environ.get("K_NJUNK", "0"))
        if NJUNK:
            junk_ps = csps.tile([64, 512], F32, tag="cs", bufs=1, name="junk_ps")
            for jj in range(NJUNK):
                nc.tensor.matmul(junk_ps[0:1, :], ones_bf[:, :],
                                 emb[0][:, 0:512].bitcast(BF16),
                                 start=True, stop=True)
        nc.gpsimd.collective_compute(
            "AllGather", ALU.bypass,
            replica_groups=[list(range(NCORES))],
            ins=[cc_in.opt()], outs=[cc_out.opt()],
        )
        zl = tailp.tile([3, NCORES * BPC], F32, name="zl")
        nc.sync.dma_start(zl[:], bass.AP(cc_out[:].tensor, 0, [[1, 3], [3, NCORES * BPC]]))
        zk = tailp.tile([3, 1], F32, name="zk")
        nc.vector.reduce_sum(zk[:], zl[:], axis=mybir.AxisListType.X)
        zcol_ps = otp[0:NU, 248:249]
        mmz = nc.tensor.matmul(zcol_ps, ksel3[:], zk[:], start=True, stop=True)
        zr = tailp.tile([NU, 1], F32)
        nc.vector.reciprocal(zr[:], zcol_ps)
        M2 = tailp.tile([NU, 64], F32)
        nc.vector.tensor_scalar_mul(M2[:], M[:], zr[:])
        rows_ps = otp[0:BPC, 96:160]
        nc.tensor.matmul(rows_ps, selT[:], M2[:], start=True, stop=True)
        if int(_os.environ.get("K_BCDMA", "1")):
            rep32 = tailp.tile([BPC, 512], F32)
            nc.vector.tensor_copy(
                rep32[:].rearrange("p (r d) -> p r d", r=8),
                rows_ps[:, None, :].broadcast_to([BPC, 8, 64]),
            )
            if int(_os.environ.get("K_ONEDMA", "0")):
                nc.sync.dma_start(
                    out.rearrange("b (p r) d -> b p (r d)", p=128),
                    rep32[:, None, :].broadcast_to([BPC, 128, 512]),
                )
            else:
                for b in range(BPC):
                    nc.sync.dma_start(
                        out[b].rearrange("(p r) d -> p (r d)", p=128)[None, :, :],
                        rep32[b:b + 1, None, :].broadcast_to([1, 128, 512]),
                    )
        else:
            rows_sb = tailp.tile([BPC, 64], FP16)
            nc.vector.tensor_copy(rows_sb[:], rows_ps)
            rep = tailp.tile([BPC, 512], FP16)
            nc.vector.tensor_copy(
                rep[:].rearrange("p (r d) -> p r d", r=8),
                rows_sb[:, None, :].broadcast_to([BPC, 8, 64]),
            )
            for b in range(BPC):
                bc_ps = csps.tile([128, 512], F32, tag="cs", bufs=1, name=f"bc_ps{b}")
                nc.tensor.matmul(bc_ps[:], oh[:, 128 * b:128 * (b + 1)], rep[:],
                                 start=True, stop=True)
                bc_sb = tailp.tile([128, 512], F32, tag=f"bc{b}")
                if b % 2 == 0:
                    nc.vector.tensor_copy(bc_sb[:], bc_ps[:])
                else:
                    nc.scalar.copy(bc_sb[:], bc_ps[:])
                nc.sync.dma_start(
                    out[b].rearrange("(p r) d -> p (r d)", p=128),
                    bc_sb[:],
                )
        ctx.close()
    nc.finalize()
    return nc


def make_in_maps(a_emb, v_emb, l_emb, fc1_w, fc1_b, fc2_w):
    embs = [a_emb, v_emb, l_emb]
    fc1T = np.ascontiguousarray(fc1_w.T, np.float32)
    fc1b = np.ascontiguousarray(fc1_b.reshape(64, 1), np.float32)
    fc2T = np.ascontiguousarray(fc2_w.T, np.float32)
    in_maps = []
    for core in range(NCORES):
        emb16 = np.zeros((3, 128, 2048), np.float16)
        nb16 = np.zeros((3, 128, 2048), np.float16)
        f0g0 = np.zeros((3, 128, 64), np.float32)
        for u in range(NU):
            p, t = u // 4, u % 4
            b = BPC * core + u // 3
            fi, gi = PAIRS[u % 3]
            f = np.asarray(embs[fi][b], np.float32)   # [1024, 32]
            g = np.asarray(embs[gi][b], np.float32)
            rb = 32 * t
            emb16[p, rb:rb + 32, 0:1024] = f.T.astype(np.float16)
            emb16[p, rb:rb + 32, 1024:2048] = g.T.astype(np.float16)
            fN = f.reshape(NCH, 128, D).transpose(1, 0, 2).reshape(128, NCH * D)
            gN = g.reshape(NCH, 128, D).transpose(1, 0, 2).reshape(128, NCH * D)
            nb16[p, :, 512 * t:512 * t + 256] = fN.astype(np.float16)
            nb16[p, :, 512 * t + 256:512 * t + 512] = gN.astype(np.float16)
            f0g0[p, rb, 0:32] = f[0]
            f0g0[p, rb, 32:64] = g[0]
        in_maps.append({
            "emb16": emb16, "nb16": nb16, "f0g0": f0g0,
            "fc1T": fc1T, "fc1b": fc1b, "fc2T": fc2T,
        })
    return in_maps


_PROGRAM_CACHE = {}


def _get_program(repeat=1):
    key = ("nc", repeat)
    if key not in _PROGRAM_CACHE:
        _PROGRAM_CACHE[key] = build_program(repeat)
    return _PROGRAM_CACHE[key]


def kernel(a_emb, v_emb, l_emb, fc1_w, fc1_b, fc2_w, _want_results=False):
    a_emb = np.asarray(a_emb, np.float32)
    v_emb = np.asarray(v_emb, np.float32)
    l_emb = np.asarray(l_emb, np.float32)
    fc1_w = np.asarray(fc1_w, np.float32)
    fc1_b = np.asarray(fc1_b, np.float32)
    fc2_w = np.asarray(fc2_w, np.float32)
    nc = _get_program()
    in_maps = make_in_maps(a_emb, v_emb, l_emb, fc1_w, fc1_b, fc2_w)
    res = None
    for attempt in range(3):
        try:
            res = run_bass_kernel_spmd(nc, in_maps, core_ids=list(range(NCORES)))
            break
        except Exception:
            if attempt == 2:
                raise
    assert res is not None
    outp = np.concatenate([res.results[c]["out"] for c in range(NCORES)], axis=0)
    if _want_results:
        return outp, res
    return outp


# revision 9
# speedup vs baseline: 1.0461x; 1.0035x over previous
"""Trainium2 Bass kernel for nn_AttnModel (BiAttn x3 + tiny FC + batch-softmax tile).

Contract: kernel(**inputs) takes the FULL inputs (a_emb/v_emb/l_emb [32,1024,32],
fc1_w [64,64], fc1_b [64], fc2_w [1,64]) and returns the FULL output [32,1024,64].

Sharding: data-parallel over batch across 8 cores (4 batches/core, 12
(batch,pair) "units"/core). Per unit only row 0 of each BiAttn output is
needed, which requires S = f@g^T [1024,1024], row/col sums of exp(S), and
row 0 / col 0 of exp(S):
  - S chunks [128i,1024j] via single-term fp16 matmuls (1 PE cycle/row;
    fp16 keeps ~2.4e-4 precision, same class as fp32r, half the DMA).
  - exp: ACT activation (bf16 out) for ~70% of chunks; DVE Schraudolph
    bit-trick exp (tensor_scalar S*A+B -> int16, bits == bf16(exp S),
    mean-calibrated B) for the rest. GPSIMD cannot touch PSUM, so Pool
    takes no exp work.
  - rowsums via DVE dummy tensor_scalar with accum_out (4x perf mode on
    bf16); colsum via per-chunk ones^T-bf16 matmuls accumulated in PSUM.
  - w1/w2 formed in j/i-partition layout via tiny PE transposes +
    reciprocal_approx_fast; o1/o2 via fp16 [128,1]x[128,32] matmuls.
  - tiny FC -> logits; ONE AllGather of exp(Ci) (15us cost-model constant)
    for the batch-dim softmax; Z-independent work precomputed before the
    collective; broadcast rows written as [1024,64] per batch.
Scheduling: emission is software-pipelined (colsum/rowsum of chunk c
deferred ~3 chunks; unit tails interleaved at chunk positions of the next
unit) to avoid head-of-line blocking in the in-order engine queues;
S triple-buffered in PSUM (3x2 banks + cs 1 + shared o/transpose bank 1).
"""
import numpy as np
import ml_dtypes

import concourse.bass as bass
import concourse.bacc as bacc
import concourse.tile as tile
import concourse.mybir as mybir
from concourse.bass_utils import run_bass_kernel_spmd
from concourse.tile_rust import add_dep_helper

F32 = mybir.dt.float32
F32R = mybir.dt.float32r
BF16 = mybir.dt.bfloat16
FP16 = mybir.dt.float16
I16 = mybir.dt.int16
AF = mybir.ActivationFunctionType
ALU = mybir.AluOpType

B, U, D = 32, 1024, 32
NCORES = 8
BPC = B // NCORES          # batches per core = 4
NU = 3 * BPC               # units per core = 12
NCH = U // 128             # i-chunks per unit = 8
PAIRS = [(0, 1), (0, 2), (1, 2)]

# Schraudolph exp->bf16 bit trick: bits = round(S * A + B); bf16(bits) ~ exp(S)
A_SCH = 128.0 / np.log(2.0)
B_SCH = 16256.0 - 7.365    # mean-centered on HW (round-to-nearest measured)

import os as _os
K_ACCUM = int(_os.environ.get("K_ACCUM", "0"))     # A-chunk rowsum: 1=ACT accum, 0=DVE dummy
K_NTREE = int(_os.environ.get("K_NTREE", "0"))     # units using add-tree colsum
K_NV = int(_os.environ.get("K_NV", "32"))
K_PFOLD = int(_os.environ.get("K_PFOLD", "1"))
_NFOLD = int(_os.environ.get("K_NFOLD", "2"))          # DVE Schraudolph chunk count

# ---- engine assignment tables (tuning knobs) ----
# exp engine per (unit, chunk): 'A' = ACT activation (+accum rowsum),
# 'V' = DVE Schraudolph (+DVE dummy rowsum)
_VPOS_TABLES = {
    0: [(1, 4, 6), (2, 5), (3, 7), (1, 5, 7), (2, 6), (3, 6)],
    1: [(2, 5)] * 6,
    2: [(2, 6), (3, 5), (2, 6), (3, 5), (2, 6), (3, 5)],
    3: [(1, 5), (2, 6), (3, 7), (1, 5), (2, 6), (3, 7)],
}
_VPOS = _VPOS_TABLES[int(_os.environ.get("K_VT", "0"))]
_LASTV = int(_os.environ.get("K_LASTV", "0"))
def exp_eng(u, c):
    if _LASTV and u >= NU - _LASTV:
        return 'V' if c in (1, 3, 5, 7) else 'A'
    base3 = K_NV // 12
    extra = K_NV - 12 * base3
    nv = base3 + (1 if u % 12 < extra else 0)
    pos = _VPOS[u % len(_VPOS)]
    return 'V' if c in pos[:nv] else 'A'

# colsum route per unit: 'PE' = per-chunk ones-matmuls accumulated in PSUM,
# 'T' = bf16 add-tree (DVE/Pool) + one ones-matmul on the Esum
def cs_route(u):
    step = 12.0 / max(K_NTREE, 1e-9)
    marks = {int(i * step) for i in range(K_NTREE)}
    return 'T' if u in marks else 'PE'

# add-tree engines: level1 pairs, level2 pairs, level3
_POOL_ADDS = int(_os.environ.get("K_POOLADDS", "0"))
_EB = int(_os.environ.get("K_EB", "4"))
_DLAG = int(_os.environ.get("K_DLAG", "3"))
_TW = [int(x) for x in _os.environ.get("K_TW", "2,5,6,7").split(",")]
def add_eng(u, lvl):
    return 'P' if lvl < _POOL_ADDS else 'V'


def build_program(repeat=1):
    nc = bacc.Bacc("TRN2", target_bir_lowering=False, debug=False, num_devices=NCORES)

    emb16 = nc.dram_tensor("emb16", [3, 128, 2048], FP16, kind="ExternalInput")
    nb16 = nc.dram_tensor("nb16", [3, 128, 2048], FP16, kind="ExternalInput")
    f0g0 = nc.dram_tensor("f0g0", [3, 128, 64], F32, kind="ExternalInput")
    fc1T = nc.dram_tensor("fc1T", [64, 64], F32, kind="ExternalInput")
    fc1b = nc.dram_tensor("fc1b", [64, 1], F32, kind="ExternalInput")
    fc2T = nc.dram_tensor("fc2T", [64, 1], F32, kind="ExternalInput")
    out = nc.dram_tensor("out", [BPC, U, 2 * D], F32, kind="ExternalOutput")

    ident_np = np.eye(128, dtype=np.float32)
    sel_np = np.zeros((NU, BPC), np.float32)
    for r in range(NU):
        sel_np[r, r // 3] = 1.0
    oh_np = np.zeros((BPC, BPC * 128), np.float16)
    for b in range(BPC):
        oh_np[b, 128 * b:128 * (b + 1)] = 1.0
    ksel_np = np.zeros((3, NU), np.float32)
    for r in range(NU):
        ksel_np[r % 3, r] = 1.0

    with tile.TileContext(nc) as tc:
        from contextlib import ExitStack
        ctx = ExitStack()
        consts = ctx.enter_context(tc.tile_pool(name="consts", bufs=1))
        bigp = ctx.enter_context(tc.tile_pool(name="big", bufs=1))
        epool = ctx.enter_context(tc.tile_pool(name="epool", bufs=1))
        upool = ctx.enter_context(tc.tile_pool(name="upool", bufs=2))
        tailp = ctx.enter_context(tc.tile_pool(name="tailp", bufs=1))
        dramp = ctx.enter_context(tc.tile_pool(name="dramp", bufs=1, space="DRAM"))

        sps = ctx.enter_context(tc.tile_pool(name="sps", bufs=1, space="PSUM"))
        csps = ctx.enter_context(tc.tile_pool(name="csps", bufs=1, space="PSUM"))
        otpp = ctx.enter_context(tc.tile_pool(name="otpp", bufs=1, space="PSUM"))

        # ---------------- input DMAs (first-needed first) ----------------
        emb, d_emb = [], []
        for p in range(3):
            t = bigp.tile([128, 2048], FP16, tag=f"emb{p}", name=f"emb_{p}")
            if p == 0:
                d0 = nc.sync.dma_start(t[0:32, :], emb16[p, 0:32, :])
                nc.sync.dma_start(t[32:64, :], emb16[p, 32:64, :])
                nc.sync.dma_start(t[64:128, :], emb16[p, 64:128, :])
                d_emb.append(d0)
            else:
                d_emb.append(nc.sync.dma_start(t[:], emb16[p, :, :]))
            emb.append(t)

        # ---------------- constants (memsets) + data DMAs in first-use order ----
        ones_bf = consts.tile([128, 1], BF16)
        nc.vector.memset(ones_bf[:], 1.0)
        ones_f = consts.tile([128, 1], F32)
        nc.vector.memset(ones_f[:], 1.0)

        nb, d_nb, fg, d_fg = [], [], [], []
        for p in range(3):
            t = bigp.tile([128, 2048], FP16, tag=f"nb{p}", name=f"nb_{p}")
            d_nb.append(nc.sync.dma_start(t[:], nb16[p, :, :]))
            nb.append(t)
            t = bigp.tile([128, 64], F32, tag=f"fg{p}")
            d_fg.append(nc.sync.dma_start(t[:], f0g0[p, :, :]))
            fg.append(t)

        ident = consts.tile([128, 128], F32)
        d_ident = nc.sync.dma_start(ident[:], nc.inline_tensor(ident_np, name="c_ident")[:, :])
        selT = consts.tile([NU, BPC], F32)
        d_sel = nc.sync.dma_start(selT[:], nc.inline_tensor(sel_np, name="c_sel")[:, :])
        oh = consts.tile([BPC, BPC * 128], FP16)
        d_oh = nc.sync.dma_start(oh[:], nc.inline_tensor(oh_np, name="c_oh")[:, :])
        fc1T_sb = consts.tile([64, 64], F32)
        d_fc1 = nc.sync.dma_start(fc1T_sb[:], fc1T[:, :])
        fc1b_sb = consts.tile([64, 1], F32)
        d_fc1b = nc.sync.dma_start(fc1b_sb[:], fc1b[:, :])
        fc2T_sb = consts.tile([64, 1], F32)
        d_fc2 = nc.sync.dma_start(fc2T_sb[:], fc2T[:, :])
        ksel3 = consts.tile([3, NU], F32)
        d_ksel = nc.sync.dma_start(ksel3[:], nc.inline_tensor(ksel_np, name="c_ksel")[:, :])

        def guard(eng, deps):
            deps = [d for d in deps if d is not None]
            if not deps:
                return None
            n = eng.nop(nofuse=True)
            for d in deps:
                add_dep_helper(n.ins, d.ins, sync=True, reason="wait-carrier")
            return n

        def pin(inst, g):
            if g is not None:
                add_dep_helper(inst.ins, g.ins, sync=False, reason="order")

        biT_sb = tailp.tile([64, NU], F32)

        # one PSUM bank shared by o-accumulation (cols 0:64), e0 transposes
        # (64:80), cs transposes (80:96), bi transpose + FC tail (96:256)
        otp = otpp.tile([128, 256], F32, tag="otp", bufs=1, name="otp")

        # ---------------- per-unit state ----------------
        class Unit:
            pass

        units = []
        for u in range(NU):
            st = Unit()
            st.u = u
            st.p, st.t = u // 4, u % 4
            st.E = None
            st.rs = None
            units.append(st)

        from collections import deque
        defer2 = deque()

        def flush_defer(limit):
            while defer2 and defer2[0][0] <= limit:
                defer2.popleft()[1]()

        def emit_chunks(u):
            """S matmuls + exp + rowsum + tree adds for unit u."""
            st = units[u]
            p, t = st.p, st.t
            rb = 32 * t
            E = epool.tile([128, NCH * 1024], BF16, tag=f"E{u % _EB}", name=f"E_{u}")
            rs = upool.tile([128, NCH], F32, tag="rs", name=f"rs_{u}")
            st.E, st.rs = E, rs
            st.partials = {}
            eslc = emb[p]
            g0 = guard(nc.tensor, [d_emb[p]]) if t == 0 else None
            st.cs_ps = csps.tile([64, 512], F32, tag="cs", bufs=1, name=f"cs_{u}")
            st.fold = K_PFOLD and u < NU - 1 and cs_route(u) == 'PE'
            st.fold3 = st.fold and int(_os.environ.get("K_VFOLD", "0"))
            st.cs_emitted = 0
            st.cs_total = NCH - (_NFOLD if st.fold else 0) - (1 if st.fold3 else 0)
            for c in range(NCH):
                S_ps = sps.tile([128, 1024], F32, tag=f"S{(8 * u + c) % 3}", name=f"S_{u}_{c}")
                for h in range(2):
                    mm = nc.tensor.matmul(
                        S_ps[:, 512 * h:512 * (h + 1)],
                        eslc[rb:rb + 32, 128 * c:128 * (c + 1)],
                        eslc[rb:rb + 32, 1024 + 512 * h:1024 + 512 * (h + 1)],
                        start=True, stop=True, tile_position=(rb, 0))
                    if c == 0 and h == 0:
                        pin(mm, g0)
                # deferred work from 2 chunks ago keeps in-order queues unblocked
                while defer2 and defer2[0][0] <= 8 * u + c:
                    defer2.popleft()[1]()
                if u > 0:
                    if c == _TW[0]:
                        emit_wcols_a(u - 1)
                    elif c == _TW[1]:
                        emit_wcols_b(u - 1)
                    if c == _TW[2]:
                        emit_omm(u - 1)
                    if c == _TW[3] and (u - 1) % 4 == 3:
                        emit_pack_bi((u - 1) // 4)
                ec = E[:, 1024 * c:1024 * (c + 1)]
                if exp_eng(u, c) == 'A':
                    if K_ACCUM:
                        nc.scalar.activation(ec, S_ps[:], AF.Exp, accum_out=rs[:, c:c + 1])
                    else:
                        nc.scalar.activation(ec, S_ps[:], AF.Exp)
                else:
                    nc.vector.tensor_scalar(ec.bitcast(I16), S_ps[:], A_SCH, B_SCH,
                                            ALU.mult, ALU.add)

                def later(u=u, c=c, st=st, ec=ec, E=E, rs=rs):
                    if exp_eng(u, c) == 'V' or not K_ACCUM:
                        dummy = upool.tile([128, 1024], BF16, tag="dum", name=f"dum_{u}_{c}")
                        nc.vector.tensor_scalar(dummy[:], ec, 1.0, 0.0, ALU.mult, ALU.add,
                                                accum_out=rs[:, c:c + 1])
                    # colsum contribution of chunk c
                    def cs_mm(ap, st=st):
                        first = st.cs_emitted == 0
                        st.cs_emitted += 1
                        last = st.cs_emitted == st.cs_total
                        for h in range(2):
                            nc.tensor.matmul(st.cs_ps[32 * h:32 * h + 1, :], ones_bf[:, :],
                                             ap[:, 512 * h:512 * (h + 1)],
                                             start=first, stop=last,
                                             tile_position=(0, 32 * h))
                    if cs_route(u) == 'PE':
                        if st.fold and c <= 2 * _NFOLD - 1:
                            # Pool pre-folds chunk pairs 0+1, 2+3 in SBUF bf16
                            if c % 2 == 1:
                                pf = upool.tile([128, 1024], BF16, tag=f"PF{c // 2}",
                                                name=f"PF{c // 2}_{u}")
                                nc.gpsimd.tensor_add(pf[:], E[:, 1024 * (c - 1):1024 * c], ec)
                                defer2.append((8 * u + c + _DLAG + 3,
                                               lambda pf=pf: cs_mm(pf[:])))
                        elif st.fold3 and c in (4, 5):
                            # third pair folded on DVE (2x bf16 add)
                            if c == 5:
                                pf = upool.tile([128, 1024], BF16, tag="PF2",
                                                name=f"PF2_{u}")
                                nc.vector.tensor_add(pf[:], E[:, 4096:5120], ec)
                                defer2.append((8 * u + c + _DLAG + 3,
                                               lambda pf=pf: cs_mm(pf[:])))
                        else:
                            cs_mm(ec)
                    else:
                        # bf16 add tree: P0..P3 = pairs, Q0,Q1, ES; adds on DVE/Pool
                        if c % 2 == 1:
                            l1 = c // 2
                            pl = upool.tile([128, 1024], BF16, tag=f"P{l1}", name=f"P{l1}_{u}")
                            eng = nc.gpsimd if add_eng(u, l1) == 'P' else nc.vector
                            eng.tensor_add(pl[:], E[:, 1024 * (c - 1):1024 * c], ec)
                            st.partials[f"P{l1}"] = pl
                            if l1 % 2 == 1:
                                l2 = l1 // 2
                                ql = upool.tile([128, 1024], BF16, tag=f"Q{l2}", name=f"Q{l2}_{u}")
                                eng = nc.gpsimd if add_eng(u, 4 + l2) == 'P' else nc.vector
                                eng.tensor_add(ql[:], st.partials[f"P{l1 - 1}"][:], pl[:])
                                st.partials[f"Q{l2}"] = ql
                        if c == NCH - 1:
                            es = upool.tile([128, 1024], BF16, tag="ES", name=f"ES_{u}")
                            eng = nc.gpsimd if add_eng(u, 6) == 'P' else nc.vector
                            eng.tensor_add(es[:], st.partials["Q0"][:], st.partials["Q1"][:])
                            for h in range(2):
                                nc.tensor.matmul(st.cs_ps[32 * h:32 * h + 1, :], ones_bf[:, :],
                                                 es[:, 512 * h:512 * (h + 1)],
                                                 start=True, stop=True,
                                                 tile_position=(0, 32 * h))
                    # e0 transposes: E row 0 (chunk 0 cols) -> j-partition columns
                    if c == 0:
                        e0v = otp[:, 64:80].bitcast(BF16)
                        for bblk in range(NCH):
                            nc.tensor.transpose(e0v[:, 2 * bblk:2 * bblk + 1],
                                                E[0:1, 128 * bblk:128 * (bblk + 1)],
                                                ones_bf[0:1, 0:1])
                        e0c = upool.tile([128, NCH], BF16, tag="e0c", name=f"e0c_{u}")
                        nc.vector.tensor_copy(e0c[:], e0v[:, 0:2 * NCH:2])
                        st.e0c = e0c
                defer2.append((8 * u + c + _DLAG, later))

        def emit_wcols_a(u):
            st = units[u]
            cs_sb = upool.tile([64, 512], F32, tag="cssb", name=f"cssb_{u}")
            nc.vector.tensor_copy(cs_sb[0:1, :], st.cs_ps[0:1, :])
            nc.scalar.copy(cs_sb[32:33, :], st.cs_ps[32:33, :])
            st.cs_sb = cs_sb
            # rowsums -> w2
            rsr = upool.tile([128, NCH], F32, tag="rsr", name=f"rsr_{u}")
            nc.vector.reciprocal_approx_fast(rsr[:], st.rs[:])
            w2 = upool.tile([128, NCH], FP16, tag="w2", name=f"w2_{u}")
            nc.vector.tensor_mul(w2[:], st.E[:, 0:NCH * 1024:1024], rsr[:])
            st.w2 = w2

        def emit_wcols_b(u):
            st = units[u]
            cs_sb = st.cs_sb
            cstp = otp[:, 80:96]
            for h in range(2):
                for bblk in range(4):
                    nc.tensor.transpose(
                        cstp[:, 4 * h + bblk:4 * h + bblk + 1],
                        cs_sb[32 * h:32 * h + 1, 128 * bblk:128 * (bblk + 1)],
                        ones_f[32 * h:32 * h + 1, 0:1])
            # cstp col order: j = 512h + 128b + part = chunk-major col (c = 4h+b)
            crec = upool.tile([128, NCH], F32, tag="crec", name=f"crec_{u}")
            nc.vector.reciprocal_approx_fast(crec[:], cstp[:, 0:NCH])
            w1 = upool.tile([128, NCH], FP16, tag="w1", name=f"w1_{u}")
            nc.vector.tensor_mul(w1[:], st.e0c[:], crec[:])
            st.w1 = w1

        def emit_omm(u):
            st = units[u]
            p, t = st.p, st.t
            o_ps = otp
            for c in range(NCH):
                # o1 = sum_j w1_j g_j   (gN at cols 512t+256..512)
                nc.tensor.matmul(
                    o_ps[32 * t:32 * t + 1, 0:32],
                    st.w1[:, c:c + 1],
                    nb[p][:, 512 * t + 256 + 32 * c:512 * t + 256 + 32 * (c + 1)],
                    start=(c == 0), stop=(c == NCH - 1), tile_position=(0, 32 * t))
            for c in range(NCH):
                # o2 = sum_i w2_i f_i   (fN at cols 512t..256)
                nc.tensor.matmul(
                    o_ps[32 * t:32 * t + 1, 32:64],
                    st.w2[:, c:c + 1],
                    nb[p][:, 512 * t + 32 * c:512 * t + 32 * (c + 1)],
                    start=(c == 0), stop=(c == NCH - 1), tile_position=(0, 32 * t))

        def emit_pack_bi(p):
            bi_rows = upool.tile([128, 64], F32, tag="bi", name=f"bi_{p}")
            nc.vector.tensor_mul(bi_rows[:], otp[:, 0:64], fg[p][:])
            tpb_ps = otp[0:64, 96:224]
            nc.tensor.transpose(tpb_ps, bi_rows[:, 0:64], ident[:])
            nc.vector.tensor_copy(biT_sb[:, 4 * p:4 * (p + 1)], tpb_ps[:, 0:97:32])

        # ---------------- main pipeline ----------------
        # tail pieces of unit u-1 are emitted at chunk positions of unit u
        for u in range(NU):
            emit_chunks(u)
        flush_defer(10 ** 9)
        emit_wcols_a(NU - 1)
        emit_wcols_b(NU - 1)
        emit_omm(NU - 1)
        emit_pack_bi(2)

        # ---------------- tail: FC + single AllGather + batch softmax ----------------
        gt1 = guard(nc.tensor, [d_fc1, d_fc1b, d_fc2, d_sel, d_oh, d_ksel, d_ident])
        hT_ps = otp[0:64, 160:172]
        mmh = nc.tensor.matmul(hT_ps, fc1T_sb[:], biT_sb[:], start=True, stop=True)
        pin(mmh, gt1)
        htT = tailp.tile([64, NU], F32)
        nc.scalar.activation(htT[:], hT_ps, AF.Tanh, bias=fc1b_sb[:, 0:1])
        ci_ps = otp[0:1, 176:188]
        nc.tensor.matmul(ci_ps, fc2T_sb[:], htT[:], start=True, stop=True)
        eci_r = tailp.tile([1, NU], F32)
        nc.scalar.activation(eci_r[:], ci_ps, AF.Exp)
        # eci as a [12,1] column for the M product (off the collective path)
        eciT_ps = otp[0:NU, 190:191]
        nc.tensor.transpose(eciT_ps, eci_r[:], ones_f[0:1, 0:1])
        eci = tailp.tile([NU, 1], F32)
        nc.vector.tensor_copy(eci[:], eciT_ps)

        # Z-independent pre-collective work: bi12 rows and M = eci * Bi
        bi12_ps = otp[0:NU, 180:244]
        nc.tensor.transpose(bi12_ps, biT_sb[:], ident[0:64, 0:64])
        bi12 = tailp.tile([NU, 64], F32)
        nc.vector.tensor_copy(bi12[:], bi12_ps)
        M = tailp.tile([NU, 64], F32)
        nc.vector.tensor_scalar_mul(M[:], bi12[:], eci[:])

        cc_in = dramp.tile([1, NU], F32, name="cc_in")
        cc_out = dramp.tile([1, NCORES * NU], F32, name="cc_out")
        nc.sync.dma_start(cc_in[:], eci_r[:])
        # keep the PE p-state hot through the collective so the final
        # broadcast matmuls run at full clock (filler writes to a dead tile)
        NJUNK = int(_os.environ.get("K_NJUNK", "0"))
        if NJUNK:
            junk_ps = csps.tile([64, 512], F32, tag="cs", bufs=1, name="junk_ps")
            for jj in range(NJUNK):
                nc.tensor.matmul(junk_ps[0:1, :], ones_bf[:, :],
                                 emb[0][:, 0:512].bitcast(BF16),
                                 start=True, stop=True)
        nc.gpsimd.collective_compute(
            "AllGather", ALU.bypass,
            replica_groups=[list(range(NCORES))],
            ins=[cc_in.opt()], outs=[cc_out.opt()],
        )
        zl = tailp.tile([3, NCORES * BPC], F32, name="zl")
        nc.sync.dma_start(zl[:], bass.AP(cc_out[:].tensor, 0, [[1, 3], [3, NCORES * BPC]]))
        zk = tailp.tile([3, 1], F32, name="zk")
        nc.vector.reduce_sum(zk[:], zl[:], axis=mybir.AxisListType.X)
        zcol_ps = otp[0:NU, 248:249]
        mmz = nc.tensor.matmul(zcol_ps, ksel3[:], zk[:], start=True, stop=True)
        zr = tailp.tile([NU, 1], F32)
        nc.vector.reciprocal(zr[:], zcol_ps)
        M2 = tailp.tile([NU, 64], F32)
        nc.vector.tensor_scalar_mul(M2[:], M[:], zr[:])
        rows_ps = otp[0:BPC, 96:160]
        nc.tensor.matmul(rows_ps, selT[:], M2[:], start=True, stop=True)
        if int(_os.environ.get("K_BCDMA", "1")):
            rep32 = tailp.tile([BPC, 512], F32)
            nc.vector.tensor_copy(
                rep32[:].rearrange("p (r d) -> p r d", r=8),
                rows_ps[:, None, :].broadcast_to([BPC, 8, 64]),
            )
            if int(_os.environ.get("K_ONEDMA", "0")):
                nc.sync.dma_start(
                    out.rearrange("b (p r) d -> b p (r d)", p=128),
                    rep32[:, None, :].broadcast_to([BPC, 128, 512]),
                )
            else:
                for b in range(BPC):
                    nc.sync.dma_start(
                        out[b].rearrange("(p r) d -> p (r d)", p=128)[None, :, :],
                        rep32[b:b + 1, None, :].broadcast_to([1, 128, 512]),
                    )
        else:
            rows_sb = tailp.tile([BPC, 64], FP16)
            nc.vector.tensor_copy(rows_sb[:], rows_ps)
            rep = tailp.tile([BPC, 512], FP16)
            nc.vector.tensor_copy(
                rep[:].rearrange("p (r d) -> p r d", r=8),
                rows_sb[:, None, :].broadcast_to([BPC, 8, 64]),
            )
            for b in range(BPC):
                bc_ps = csps.tile([128, 512], F32, tag="cs", bufs=1, name=f"bc_ps{b}")
                nc.tensor.matmul(bc_ps[:], oh[:, 128 * b:128 * (b + 1)], rep[:],
                                 start=True, stop=True)
                bc_sb = tailp.tile([128, 512], F32, tag=f"bc{b}")
                if b % 2 == 0:
                    nc.vector.tensor_copy(bc_sb[:], bc_ps[:])
                else:
                    nc.scalar.copy(bc_sb[:], bc_ps[:])
                nc.sync.dma_start(
                    out[b].rearrange("(p r) d -> p (r d)", p=128),
                    bc_sb[:],
                )
        ctx.close()
    nc.finalize()
    return nc


def make_in_maps(a_emb, v_emb, l_emb, fc1_w, fc1_b, fc2_w):
    embs = [a_emb, v_emb, l_emb]
    fc1T = np.ascontiguousarray(fc1_w.T, np.float32)
    fc1b = np.ascontiguousarray(fc1_b.reshape(64, 1), np.float32)
    fc2T = np.ascontiguousarray(fc2_w.T, np.float32)
    in_maps = []
    for core in range(NCORES):
        emb16 = np.zeros((3, 128, 2048), np.float16)
        nb16 = np.zeros((3, 128, 2048), np.float16)
        f0g0 = np.zeros((3, 128, 64), np.float32)
        for u in range(NU):
            p, t = u // 4, u % 4
            b = BPC * core + u // 3
            fi, gi = PAIRS[u % 3]
            f = np.asarray(embs[fi][b], np.float32)   # [1024, 32]
            g = np.asarray(embs[gi][b], np.float32)
            rb = 32 * t
            emb16[p, rb:rb + 32, 0:1024] = f.T.astype(np.float16)
            emb16[p, rb:rb + 32, 1024:2048] = g.T.astype(np.float16)
            fN = f.reshape(NCH, 128, D).transpose(1, 0, 2).reshape(128, NCH * D)
            gN = g.reshape(NCH, 128, D).transpose(1, 0, 2).reshape(128, NCH * D)
            nb16[p, :, 512 * t:512 * t + 256] = fN.astype(np.float16)
            nb16[p, :, 512 * t + 256:512 * t + 512] = gN.astype(np.float16)
            f0g0[p, rb, 0:32] = f[0]
            f0g0[p, rb, 32:64] = g[0]
        in_maps.append({
            "emb16": emb16, "nb16": nb16, "f0g0": f0g0,
            "fc1T": fc1T, "fc1b": fc1b, "fc2T": fc2T,
        })
    return in_maps


_PROGRAM_CACHE = {}


def _get_program(repeat=1):
    key = ("nc", repeat)
    if key not in _PROGRAM_CACHE:
        _PROGRAM_CACHE[key] = build_program(repeat)
    return _PROGRAM_CACHE[key]


def kernel(a_emb, v_emb, l_emb, fc1_w, fc1_b, fc2_w, _want_results=False):
    a_emb = np.asarray(a_emb, np.float32)
    v_emb = np.asarray(v_emb, np.float32)
    l_emb = np.asarray(l_emb, np.float32)
    fc1_w = np.asarray(fc1_w, np.float32)
    fc1_b = np.asarray(fc1_b, np.float32)
    fc2_w = np.asarray(fc2_w, np.float32)
    nc = _get_program()
    in_maps = make_in_maps(a_emb, v_emb, l_emb, fc1_w, fc1_b, fc2_w)
    res = None
    for attempt in range(3):
        try:
            res = run_bass_kernel_spmd(nc, in_maps, core_ids=list(range(NCORES)))
            break
        except Exception:
            if attempt == 2:
                raise
    assert res is not None
    outp = np.concatenate([res.results[c]["out"] for c in range(NCORES)], axis=0)
    if _want_results:
        return outp, res
    return outp
